# revision 11
# baseline (speedup 1.0000x reference)
"""MinGRU layer Trainium2 kernel.

Reference semantics (B=8, T=16384, D=H=O=256):
    zs = sigmoid(xs @ Wz.T + bz);  hs = xs @ Wh.T + bh
    a = concat([1], 1-zs);  b = concat([0], zs*hs)         (T+1 positions)
    states = jax.lax.associative_scan(combine, (a, b))[1][:, 1:]
    out = states @ Wo.T + bo
with combine((a0,b0),(a1,b1)) = (a0*b0, b0*a1 + b1).

The combine is NOT associative, so the result is defined by jax's exact
odd/even recursion tree (Blelloch-style).  We replicate that tree exactly:
  - positions split into 8 aligned chunks of L=2048 (+1 trailing position);
  - per-chunk bottom-up reduce ("up-sweep", keeping all tree levels);
  - a tiny cross-chunk scan over the 8 chunk-top elements following the same
    recursion (gives each chunk's prefix-in value and chunk-end outputs);
  - per-chunk top-down "down-sweep" filling every position's scan value.

Sharding: batch b=8 across the 8 cores (1 sequence per core); weights
replicated.  The host pre-transposes/casts x and the weights so the device
only does: DMA -> matmul (bf16) -> sigmoid (ACT) -> scan tree (DVE/f32) ->
output matmul (bf16) -> DMA.  Per-core output is [O, T]; host transposes.
"""

from contextlib import ExitStack

import numpy as np
import ml_dtypes

import concourse.bass as bass
import concourse.bacc as bacc
import concourse.tile as tile
from concourse import mybir
from concourse.bass_utils import run_bass_kernel_spmd

BF16 = ml_dtypes.bfloat16
F32 = mybir.dt.float32
BF = mybir.dt.bfloat16

B, T, D, H, O = 8, 16384, 256, 256, 256
L = 2048          # positions per chunk (power of 2)
NCHUNK = T // L   # 8 full chunks; position T (=16384) handled separately
SUB = 512         # matmul sub-chunk (fits one PSUM bank at f32)
LMAX = 11         # log2(L)

AluOp = mybir.AluOpType
ActFn = mybir.ActivationFunctionType


def _level_offsets():
    """Offsets of levels 1..LMAX inside the packed up-sweep buffers."""
    off = {1: 0}
    n = L // 2
    for lvl in range(1, LMAX):
        off[lvl + 1] = off[lvl] + n
        n //= 2
    total = off[LMAX] + 1
    return off, total


LVL_OFF, LVL_TOTAL = _level_offsets()  # total = 2047


def build_nc():
    nc = bacc.Bacc()

    xt = nc.dram_tensor("xt", [D, T], mybir.dt.bfloat16, kind="ExternalInput")
    wzt = nc.dram_tensor("wzt", [D, H], mybir.dt.bfloat16, kind="ExternalInput")
    wht = nc.dram_tensor("wht", [D, H], mybir.dt.bfloat16, kind="ExternalInput")
    wot = nc.dram_tensor("wot", [H, O], mybir.dt.bfloat16, kind="ExternalInput")
    bzp = nc.dram_tensor("bzp", [H, 1], F32, kind="ExternalInput")   # +bz
    bzn = nc.dram_tensor("bzn", [H, 1], F32, kind="ExternalInput")   # -bz
    bhb = nc.dram_tensor("bhb", [H, 1], F32, kind="ExternalInput")
    bob = nc.dram_tensor("bob", [O, 1], F32, kind="ExternalInput")
    out = nc.dram_tensor("out", [O, T], F32, kind="ExternalOutput")

    with tile.TileContext(nc) as tc, ExitStack() as ctx:
        singles = ctx.enter_context(tc.tile_pool(name="singles", bufs=1))
        ab_pool = ctx.enter_context(tc.tile_pool(name="ab", bufs=2))
        lvl_pool = ctx.enter_context(tc.tile_pool(name="lvl", bufs=1))
        dbuf_pool = ctx.enter_context(tc.tile_pool(name="dbuf", bufs=2))
        st_pool = ctx.enter_context(tc.tile_pool(name="st", bufs=1))
        tmp_pool = ctx.enter_context(tc.tile_pool(name="tmp", bufs=1))
        z_pool = ctx.enter_context(tc.tile_pool(name="zp", bufs=2))
        x_pool = ctx.enter_context(tc.tile_pool(name="xp", bufs=2))
        osb_pool = ctx.enter_context(tc.tile_pool(name="osb", bufs=2))
        psum_y = ctx.enter_context(tc.tile_pool(name="psy", bufs=1, space="PSUM"))
        psum_o = ctx.enter_context(tc.tile_pool(name="pso", bufs=2, space="PSUM"))

        # ---- constants ----
        wz_sb = []
        wh_sb = []
        wo_sb = []
        for k in range(2):
            wzk = singles.tile([128, H], BF, name=f"wzk{k}")
            nc.sync.dma_start(out=wzk, in_=wzt[k * 128:(k + 1) * 128, :])
            wz_sb.append(wzk)
            whk = singles.tile([128, H], BF, name=f"whk{k}")
            nc.sync.dma_start(out=whk, in_=wht[k * 128:(k + 1) * 128, :])
            wh_sb.append(whk)
            wok = singles.tile([128, O], BF, name=f"wok{k}")
            nc.sync.dma_start(out=wok, in_=wot[k * 128:(k + 1) * 128, :])
            wo_sb.append(wok)
        bzp_sb, bzn_sb, bh_sb, bo_sb = [], [], [], []
        for h in range(2):
            pz = singles.tile([128, 1], F32, name=f"bzp{h}")
            nc.sync.dma_start(out=pz, in_=bzp[h * 128:(h + 1) * 128, :])
            bzp_sb.append(pz)
            nz = singles.tile([128, 1], F32, name=f"bzn{h}")
            nc.sync.dma_start(out=nz, in_=bzn[h * 128:(h + 1) * 128, :])
            bzn_sb.append(nz)
            hb = singles.tile([128, 1], F32, name=f"bh{h}")
            nc.sync.dma_start(out=hb, in_=bhb[h * 128:(h + 1) * 128, :])
            bh_sb.append(hb)
            ob = singles.tile([128, 1], F32, name=f"bo{h}")
            nc.sync.dma_start(out=ob, in_=bob[h * 128:(h + 1) * 128, :])
            bo_sb.append(ob)

        # top-level bookkeeping tiles (per half)
        tops_A = [singles.tile([128, 8], F32, name=f"topsA{h}") for h in range(2)]
        tops_B = [singles.tile([128, 8], F32, name=f"topsB{h}") for h in range(2)]
        # spine scratch: cols 0-3 sB12_0..3, 4 sA12_1, 5 sA12_2, 6 sA12_3,
        # 7 sB13_0, 8 sB13_1, 9 sA13_1, 10 sB14
        spine = [singles.tile([128, 12], F32, name=f"spine{h}") for h in range(2)]
        otb = [singles.tile([128, 8], F32, name=f"otb{h}") for h in range(2)]

        def emit_subchunk_mats(c, s, x0, ncols, acol):
            """DMA x cols [x0, x0+ncols), matmuls, sigmoid/b into
            a_buf/b_buf[...acol:acol+ncols) for chunk-local position cols."""
            xk = []
            for k in range(2):
                xkt = x_pool.tile([128, SUB], BF, name="xkt", tag=f"xkt{k}")
                nc.sync.dma_start(out=xkt[:, :ncols],
                                  in_=xt[k * 128:(k + 1) * 128, x0:x0 + ncols])
                xk.append(xkt)
            for h in range(2):
                yz = psum_y.tile([128, SUB], F32, name="yz", tag=f"yz{h}")
                yh = psum_y.tile([128, SUB], F32, name="yh", tag=f"yh{h}")
                for k in range(2):
                    nc.tensor.matmul(yz[:, :ncols],
                                     wz_sb[k][:, h * 128:(h + 1) * 128],
                                     xk[k][:, :ncols],
                                     start=(k == 0), stop=(k == 1))
                for k in range(2):
                    nc.tensor.matmul(yh[:, :ncols],
                                     wh_sb[k][:, h * 128:(h + 1) * 128],
                                     xk[k][:, :ncols],
                                     start=(k == 0), stop=(k == 1))
                zt = z_pool.tile([128, SUB], F32, name="zt", tag=f"zt{h}")
                # z = sigmoid(y_z + bz)
                nc.scalar.activation(zt[:, :ncols], yz[:, :ncols], ActFn.Sigmoid,
                                     bias=bzp_sb[h][:, 0:1], scale=1.0)
                # a = 1 - z = sigmoid(-y_z - bz)
                nc.scalar.activation(a_buf[h][:, acol:acol + ncols], yz[:, :ncols],
                                     ActFn.Sigmoid, bias=bzn_sb[h][:, 0:1],
                                     scale=-1.0)
                # b = (y_h + bh) * z
                nc.vector.scalar_tensor_tensor(
                    b_buf[h][:, acol:acol + ncols], yh[:, :ncols],
                    bh_sb[h][:, 0:1], zt[:, :ncols],
                    op0=AluOp.add, op1=AluOp.mult)

        def combine_cols(h, dstA, dstB, lA, lB, rA, rB, want_A=True):
            """(dstA, dstB) = combine((lA,lB), (rA,rB)) on [128,1] APs.
            B' = lB*rA + rB done as one scalar_tensor_tensor (lB is [128,1])."""
            if want_A:
                nc.vector.tensor_tensor(dstA, lA, lB, op=AluOp.mult)
            nc.vector.scalar_tensor_tensor(dstB, rA, lB, rB,
                                           op0=AluOp.mult, op1=AluOp.add)

        # allocate a/b and tree buffers fresh per chunk from pools (slot reuse)
        for c in range(NCHUNK):
            a_buf = [ab_pool.tile([128, L], F32, name="a_buf", tag=f"a{h}")
                     for h in range(2)]
            b_buf = [ab_pool.tile([128, L], F32, name="b_buf", tag=f"b{h}")
                     for h in range(2)]

            # ---- phase 1: matmuls + sigmoid -> a/b arrays ----
            if c == 0:
                # position 0 is the prepended (1, 0) element
                for h in range(2):
                    nc.vector.memset(a_buf[h][:, 0:1], 1.0)
                    nc.vector.memset(b_buf[h][:, 0:1], 0.0)
                for s in range(4):
                    x0 = s * SUB
                    ncols = SUB if s < 3 else SUB - 1
                    emit_subchunk_mats(c, s, x0, ncols, s * SUB + 1)
            else:
                base = c * L - 1
                for s in range(4):
                    emit_subchunk_mats(c, s, base + s * SUB, SUB, s * SUB)

            # ---- phase 2: up-sweep ----
            Aup = [lvl_pool.tile([128, LVL_TOTAL], F32, name="Aup", tag=f"Au{h}")
                   for h in range(2)]
            Bup = [lvl_pool.tile([128, LVL_TOTAL], F32, name="Bup", tag=f"Bu{h}")
                   for h in range(2)]
            for h in range(2):
                for lvl in range(LMAX):
                    n = L >> lvl
                    m = n // 2
                    if lvl == 0:
                        sA, sB = a_buf[h], b_buf[h]
                    else:
                        o = LVL_OFF[lvl]
                        sA = Aup[h][:, o:o + n]
                        sB = Bup[h][:, o:o + n]
                    o2 = LVL_OFF[lvl + 1]
                    dA = Aup[h][:, o2:o2 + m]
                    dB = Bup[h][:, o2:o2 + m]
                    A_ev, A_od = sA[:, 0:n:2], sA[:, 1:n:2]
                    B_ev, B_od = sB[:, 0:n:2], sB[:, 1:n:2]
                    nc.vector.tensor_tensor(dA, A_ev, B_ev, op=AluOp.mult)
                    tu = tmp_pool.tile([128, L // 2], F32, name="tu", tag=f"tu{h}")
                    nc.vector.tensor_tensor(tu[:, :m], B_ev, A_od, op=AluOp.mult)
                    nc.vector.tensor_tensor(dB, tu[:, :m], B_od, op=AluOp.add)

            # ---- phase 3: top-level bookkeeping ----
            o11 = LVL_OFF[LMAX]
            for h in range(2):
                EA = tops_A[h][:, c:c + 1]
                EB = tops_B[h][:, c:c + 1]
                nc.vector.tensor_copy(EA, Aup[h][:, o11:o11 + 1])
                nc.vector.tensor_copy(EB, Bup[h][:, o11:o11 + 1])
                sp = spine[h]
                tA = tops_A[h]
                tB = tops_B[h]
                ob_ = otb[h]
                cc = lambda i: (tA[:, i:i + 1], tB[:, i:i + 1])
                if c == 0:
                    nc.vector.tensor_copy(ob_[:, 0:1], EB)
                elif c == 1:
                    combine_cols(h, None, sp[:, 0:1], *cc(0), *cc(1), want_A=False)
                    nc.vector.tensor_copy(ob_[:, 1:2], sp[:, 0:1])
                elif c == 2:
                    combine_cols(h, None, ob_[:, 2:3], None, ob_[:, 1:2],
                                 EA, EB, want_A=False)
                elif c == 3:
                    combine_cols(h, None, sp[:, 1:2], *cc(2), *cc(3), want_A=False)
                    # sA12_1 = A_E2 * B_E2
                    nc.vector.tensor_tensor(sp[:, 4:5], tA[:, 2:3], tB[:, 2:3],
                                            op=AluOp.mult)
                    # sB13_0 = sB12_0 * sA12_1 + sB12_1
                    combine_cols(h, None, sp[:, 7:8], None, sp[:, 0:1],
                                 sp[:, 4:5], sp[:, 1:2], want_A=False)
                    nc.vector.tensor_copy(ob_[:, 3:4], sp[:, 7:8])
                elif c == 4:
                    combine_cols(h, None, ob_[:, 4:5], None, ob_[:, 3:4],
                                 EA, EB, want_A=False)
                elif c == 5:
                    combine_cols(h, None, sp[:, 2:3], *cc(4), *cc(5), want_A=False)
                    nc.vector.tensor_tensor(sp[:, 5:6], tA[:, 4:5], tB[:, 4:5],
                                            op=AluOp.mult)  # sA12_2
                    combine_cols(h, None, ob_[:, 5:6], None, ob_[:, 3:4],
                                 sp[:, 5:6], sp[:, 2:3], want_A=False)
                elif c == 6:
                    combine_cols(h, None, ob_[:, 6:7], None, ob_[:, 5:6],
                                 EA, EB, want_A=False)
                elif c == 7:
                    combine_cols(h, None, sp[:, 3:4], *cc(6), *cc(7), want_A=False)
                    nc.vector.tensor_tensor(sp[:, 6:7], tA[:, 6:7], tB[:, 6:7],
                                            op=AluOp.mult)  # sA12_3
                    # sB13_1 = sB12_2 * sA12_3 + sB12_3
                    combine_cols(h, None, sp[:, 8:9], None, sp[:, 2:3],
                                 sp[:, 6:7], sp[:, 3:4], want_A=False)
                    # sA13_1 = sA12_2 * sB12_2
                    nc.vector.tensor_tensor(sp[:, 9:10], sp[:, 5:6], sp[:, 2:3],
                                            op=AluOp.mult)
                    # sB14 = sB13_0 * sA13_1 + sB13_1
                    combine_cols(h, None, sp[:, 10:11], None, sp[:, 7:8],
                                 sp[:, 9:10], sp[:, 8:9], want_A=False)
                    nc.vector.tensor_copy(ob_[:, 7:8], sp[:, 10:11])

            # ---- phase 4: down-sweep ----
            dbuf = [dbuf_pool.tile([128, L + 1], F32, name="dbuf", tag=f"d{h}")
                    for h in range(2)]
            for h in range(2):
                if c == 0:
                    nc.vector.memset(dbuf[h][:, 0:1], 0.0)
                else:
                    nc.vector.tensor_copy(dbuf[h][:, 0:1], otb[h][:, c - 1:c])
                nc.vector.tensor_copy(dbuf[h][:, L:L + 1], otb[h][:, c:c + 1])
                for lvl in range(LMAX - 1, -1, -1):
                    n = L >> lvl
                    cnt = n // 2
                    step = 1 << (lvl + 1)
                    if lvl == 0:
                        A_src, B_src = a_buf[h], b_buf[h]
                    else:
                        o = LVL_OFF[lvl]
                        A_src = Aup[h][:, o:o + n]
                        B_src = Bup[h][:, o:o + n]
                    A_ev = A_src[:, 0:n:2]
                    B_ev = B_src[:, 0:n:2]
                    Lh = dbuf[h][:, 0:L:step]
                    Wt = dbuf[h][:, (1 << lvl):L:step]
                    td = tmp_pool.tile([128, L // 2], F32, name="td", tag=f"td{h}")
                    nc.vector.tensor_tensor(td[:, :cnt], Lh, A_ev, op=AluOp.mult)
                    nc.vector.tensor_tensor(Wt, td[:, :cnt], B_ev, op=AluOp.add)

            # ---- phase 5: output matmul + store ----
            if c == 0:
                s0, ncols_all, obase = 2, L - 1, 0
            else:
                s0, ncols_all, obase = 1, L, c * L - 1
            st = [st_pool.tile([128, L], BF, name="st", tag=f"s{h}")
                  for h in range(2)]
            for h in range(2):
                nc.gpsimd.tensor_copy(st[h][:, :ncols_all],
                                      dbuf[h][:, s0:s0 + ncols_all])
            nsub = (ncols_all + SUB - 1) // SUB
            for s in range(nsub):
                col0 = s * SUB
                ncols = min(SUB, ncols_all - col0)
                for h in range(2):  # output half (o dimension)
                    po = psum_o.tile([128, SUB], F32, name="po", tag=f"po{h}")
                    for k in range(2):  # contraction over hidden halves
                        nc.tensor.matmul(po[:, :ncols],
                                         wo_sb[k][:, h * 128:(h + 1) * 128],
                                         st[k][:, col0:col0 + ncols],
                                         start=(k == 0), stop=(k == 1))
                    osb = osb_pool.tile([128, SUB], F32, name="osb", tag=f"o{h}")
                    nc.scalar.activation(osb[:, :ncols], po[:, :ncols],
                                         ActFn.Identity, bias=bo_sb[h][:, 0:1],
                                         scale=1.0)
                    nc.sync.dma_start(
                        out=out[h * 128:(h + 1) * 128,
                                obase + col0:obase + col0 + ncols],
                        in_=osb[:, :ncols])

            # keep handles for final position (chunk 7's dbuf)
            if c == NCHUNK - 1:
                last_dbuf = dbuf

        # ---- final position T (=16384): out[p] = out[p-1]*a + b ----
        xk1 = []
        for k in range(2):
            xl = singles.tile([128, 1], BF, name=f"xl{k}")
            nc.sync.dma_start(out=xl, in_=xt[k * 128:(k + 1) * 128, T - 1:T])
            xk1.append(xl)
        stl = []
        for h in range(2):
            yzl = psum_y.tile([128, SUB], F32, name="yzl", tag=f"yz{h}")[:, 0:1]
            yhl = psum_y.tile([128, SUB], F32, name="yhl", tag=f"yh{h}")[:, 0:1]
            for k in range(2):
                nc.tensor.matmul(yzl, wz_sb[k][:, h * 128:(h + 1) * 128],
                                 xk1[k], start=(k == 0), stop=(k == 1))
            for k in range(2):
                nc.tensor.matmul(yhl, wh_sb[k][:, h * 128:(h + 1) * 128],
                                 xk1[k], start=(k == 0), stop=(k == 1))
            zl = singles.tile([128, 1], F32, name=f"zl{h}")
            al = singles.tile([128, 1], F32, name=f"al{h}")
            bl = singles.tile([128, 1], F32, name=f"bl{h}")
            nc.scalar.activation(zl, yzl, ActFn.Sigmoid,
                                 bias=bzp_sb[h][:, 0:1], scale=1.0)
            nc.scalar.activation(al, yzl, ActFn.Sigmoid,
                                 bias=bzn_sb[h][:, 0:1], scale=-1.0)
            nc.vector.scalar_tensor_tensor(bl, yhl, bh_sb[h][:, 0:1], zl,
                                           op0=AluOp.add, op1=AluOp.mult)
            dl = singles.tile([128, 1], F32, name=f"dl{h}")
            # dl = dbuf_end * al + bl  (al is [128,1] -> scalar operand)
            nc.vector.scalar_tensor_tensor(dl, last_dbuf[h][:, L:L + 1], al, bl,
                                           op0=AluOp.mult, op1=AluOp.add)
            sl = singles.tile([128, 1], BF, name=f"sl{h}")
            nc.gpsimd.tensor_copy(sl, dl)
            stl.append(sl)
        for h in range(2):
            pol = psum_o.tile([128, SUB], F32, name="pol", tag=f"po{h}")[:, 0:1]
            for k in range(2):
                nc.tensor.matmul(pol, wo_sb[k][:, h * 128:(h + 1) * 128],
                                 stl[k], start=(k == 0), stop=(k == 1))
            osl = singles.tile([128, 1], F32, name=f"osl{h}")
            nc.scalar.activation(osl, pol, ActFn.Identity,
                                 bias=bo_sb[h][:, 0:1], scale=1.0)
            nc.sync.dma_start(out=out[h * 128:(h + 1) * 128, T - 1:T], in_=osl)

    nc.compile()
    return nc


_NC_CACHE = {}


def _get_nc():
    if "nc" not in _NC_CACHE:
        _NC_CACHE["nc"] = build_nc()
    return _NC_CACHE["nc"]


def _prepare_in_maps(xs, Wz, bz, Wh, bh, Wo, bo):
    xs = np.asarray(xs, np.float32)
    Wz = np.asarray(Wz, np.float32)
    bz = np.asarray(bz, np.float32)
    Wh = np.asarray(Wh, np.float32)
    bh = np.asarray(bh, np.float32)
    Wo = np.asarray(Wo, np.float32)
    bo = np.asarray(bo, np.float32)

    wzt = np.ascontiguousarray(Wz.T).astype(BF16)
    wht = np.ascontiguousarray(Wh.T).astype(BF16)
    wot = np.ascontiguousarray(Wo.T).astype(BF16)
    bzp = np.ascontiguousarray(bz.reshape(H, 1))
    bzn = np.ascontiguousarray((-bz).reshape(H, 1))
    bhb = np.ascontiguousarray(bh.reshape(H, 1))
    bob = np.ascontiguousarray(bo.reshape(O, 1))

    in_maps = []
    for i in range(B):
        xti = np.ascontiguousarray(xs[i].T).astype(BF16)
        in_maps.append({
            "xt": xti, "wzt": wzt, "wht": wht, "wot": wot,
            "bzp": bzp, "bzn": bzn, "bhb": bhb, "bob": bob,
        })
    return in_maps


def _assemble(res):
    return np.stack([np.asarray(res.results[i]["out"], np.float32).T
                     for i in range(B)], axis=0)


def run_traced(xs, Wz, bz, Wh, bh, Wo, bo, trace=True):
    """Run on hardware with NTFF profiling; returns (out, BassKernelResults)."""
    in_maps = _prepare_in_maps(xs, Wz, bz, Wh, bh, Wo, bo)
    res = run_bass_kernel_spmd(_get_nc(), in_maps, core_ids=list(range(B)),
                               trace=trace)
    return _assemble(res), res


def kernel(xs, Wz, bz, Wh, bh, Wo, bo):
    in_maps = _prepare_in_maps(xs, Wz, bz, Wh, bh, Wo, bo)
    res = run_bass_kernel_spmd(_get_nc(), in_maps, core_ids=list(range(B)))
    return _assemble(res)


# revision 18
# speedup vs baseline: 1.0426x; 1.0426x over previous
"""MinGRU layer Trainium2 kernel.

Reference semantics (B=8, T=16384, D=H=O=256):
    zs = sigmoid(xs @ Wz.T + bz);  hs = xs @ Wh.T + bh
    a = concat([1], 1-zs);  b = concat([0], zs*hs)         (T+1 positions)
    states = jax.lax.associative_scan(combine, (a, b))[1][:, 1:]
    out = states @ Wo.T + bo
with combine((a0,b0),(a1,b1)) = (a0*b0, b0*a1 + b1).

The combine is NOT associative, so the result is defined by jax's exact
odd/even recursion tree.  We replicate that tree exactly:
  - positions split into 8 aligned chunks of L=2048 (+1 trailing position);
  - per-chunk bottom-up reduce ("up-sweep", keeping all tree levels);
  - a tiny cross-chunk scan over the 8 chunk-top elements following the same
    recursion (chunk prefixes + chunk-end outputs);
  - per-chunk top-down "down-sweep" filling every position's scan value.

Sharding: batch b=8 across the 8 cores (one sequence per core); weights
replicated.  The host pre-transposes/casts x and the weights; the device does
DMA -> matmul (bf16) -> sigmoid (ACT) -> scan tree (DVE+GpSimd, f32, both
hidden halves fused per op via 3D APs) -> output matmul (f32r, reads the
scan buffer directly) -> PSUM DMA'd straight to DRAM.  Per-core output is
[O, T] without the output bias; the host transposes and adds bo.
"""

from contextlib import ExitStack

import numpy as np
import ml_dtypes

import concourse.bacc as bacc
import concourse.tile as tile
from concourse import mybir
from concourse.bass_utils import run_bass_kernel_spmd

BF16 = ml_dtypes.bfloat16
F32 = mybir.dt.float32
F32R = mybir.dt.float32r
BF = mybir.dt.bfloat16

B, T, D, H, O = 8, 16384, 256, 256, 256
L = 2048          # positions per chunk (power of 2)
NCHUNK = T // L   # 8 full chunks; position T (=16384) handled separately
SUB = 512         # matmul sub-chunk (one PSUM bank at f32)
LMAX = 11         # log2(L)

AluOp = mybir.AluOpType
ActFn = mybir.ActivationFunctionType


def _level_offsets():
    off = {1: 0}
    n = L // 2
    for lvl in range(1, LMAX):
        off[lvl + 1] = off[lvl] + n
        n //= 2
    return off, off[LMAX] + 1


LVL_OFF, LVL_TOTAL = _level_offsets()  # total = 2047


def build_nc():
    nc = bacc.Bacc()

    xt = nc.dram_tensor("xt", [D, T], BF, kind="ExternalInput")
    wzt = nc.dram_tensor("wzt", [D, H], BF, kind="ExternalInput")
    wht = nc.dram_tensor("wht", [D, H], BF, kind="ExternalInput")
    wot = nc.dram_tensor("wot", [H, O], F32R, kind="ExternalInput")
    bzp = nc.dram_tensor("bzp", [H, 1], F32, kind="ExternalInput")   # +bz
    bzn = nc.dram_tensor("bzn", [H, 1], F32, kind="ExternalInput")   # -bz
    bhb = nc.dram_tensor("bhb", [H, 1], F32, kind="ExternalInput")
    out = nc.dram_tensor("out", [O, T], F32, kind="ExternalOutput")

    with tile.TileContext(nc) as tc, ExitStack() as ctx:
        singles = ctx.enter_context(tc.tile_pool(name="singles", bufs=1))
        ab_pool = ctx.enter_context(tc.tile_pool(name="ab", bufs=2))
        lvl_pool = ctx.enter_context(tc.tile_pool(name="lvl", bufs=1))
        dbuf_pool = ctx.enter_context(tc.tile_pool(name="dbuf", bufs=2))
        tmp_pool = ctx.enter_context(tc.tile_pool(name="tmp", bufs=2))
        z_pool = ctx.enter_context(tc.tile_pool(name="zp", bufs=2))
        x_pool = ctx.enter_context(tc.tile_pool(name="xp", bufs=3))
        osb_pool = ctx.enter_context(tc.tile_pool(name="osb", bufs=3))
        psum_y = ctx.enter_context(tc.tile_pool(name="psy", bufs=2, space="PSUM"))
        psum_o = ctx.enter_context(tc.tile_pool(name="pso", bufs=2, space="PSUM"))

        # ---- constants ----
        wz_sb, wh_sb, wo_sb = [], [], []
        for k in range(2):
            wzk = singles.tile([128, H], BF, name=f"wzk{k}")
            nc.sync.dma_start(out=wzk, in_=wzt[k * 128:(k + 1) * 128, :])
            wz_sb.append(wzk)
            whk = singles.tile([128, H], BF, name=f"whk{k}")
            nc.sync.dma_start(out=whk, in_=wht[k * 128:(k + 1) * 128, :])
            wh_sb.append(whk)
            wok = singles.tile([128, O], F32R, name=f"wok{k}")
            nc.sync.dma_start(out=wok, in_=wot[k * 128:(k + 1) * 128, :])
            wo_sb.append(wok)
        bzp_sb, bzn_sb, bh_sb = [], [], []
        for h in range(2):
            pz = singles.tile([128, 1], F32, name=f"bzp{h}")
            nc.sync.dma_start(out=pz, in_=bzp[h * 128:(h + 1) * 128, :])
            bzp_sb.append(pz)
            nz = singles.tile([128, 1], F32, name=f"bzn{h}")
            nc.sync.dma_start(out=nz, in_=bzn[h * 128:(h + 1) * 128, :])
            bzn_sb.append(nz)
            hb = singles.tile([128, 1], F32, name=f"bh{h}")
            nc.sync.dma_start(out=hb, in_=bhb[h * 128:(h + 1) * 128, :])
            bh_sb.append(hb)

        # top-level bookkeeping, both halves fused: [128, 2, n]
        tops_A = singles.tile([128, 2, 8], F32, name="topsA")
        tops_B = singles.tile([128, 2, 8], F32, name="topsB")
        # spine: 0-3 sB12_0..3, 4 sA12_1, 5 sA12_2, 6 sA12_3,
        #        7 sB13_0, 8 sB13_1, 9 sA13_1, 10 sB14
        spine = singles.tile([128, 2, 12], F32, name="spine")
        otb = singles.tile([128, 2, 8], F32, name="otb")

        def top_combine(dstB, lB, rA, rB, dstA=None, lA=None):
            """combine on [128,2,1] APs: dstB = lB*rA + rB; dstA = lA*lB."""
            if dstA is not None:
                nc.vector.tensor_tensor(dstA, lA, lB, op=AluOp.mult)
            t = tmp_pool.tile([128, 2, 1], F32, name="ttop", tag="ttop")
            nc.vector.tensor_tensor(t, lB, rA, op=AluOp.mult)
            nc.vector.tensor_tensor(dstB, t, rB, op=AluOp.add)

        def emit_subchunk_mats(x0, ncols, acol):
            """DMA x cols [x0, x0+ncols); z/h matmuls; sigmoids; b into
            a_buf/b_buf[:, :, acol:acol+ncols)."""
            xk = x_pool.tile([128, 2, SUB], BF, name="xk", tag="xk")
            nc.sync.dma_start(
                out=xk[:, :, :ncols],
                in_=xt[:, x0:x0 + ncols].rearrange("(k p) n -> p k n", p=128))
            for h in range(2):
                yz = psum_y.tile([128, SUB], F32, name="yz", tag=f"y{h}")
                for k in range(2):
                    nc.tensor.matmul(yz[:, :ncols],
                                     wz_sb[k][:, h * 128:(h + 1) * 128],
                                     xk[:, k, :ncols],
                                     start=(k == 0), stop=(k == 1))
                zt = z_pool.tile([128, SUB], F32, name="zt", tag=f"zt{h}")
                nc.scalar.activation(zt[:, :ncols], yz[:, :ncols], ActFn.Sigmoid,
                                     bias=bzp_sb[h][:, 0:1], scale=1.0)
                nc.scalar.activation(a_buf[:, h, acol:acol + ncols],
                                     yz[:, :ncols], ActFn.Sigmoid,
                                     bias=bzn_sb[h][:, 0:1], scale=-1.0)
                yh = psum_y.tile([128, SUB], F32, name="yh", tag=f"y{h}")
                for k in range(2):
                    nc.tensor.matmul(yh[:, :ncols],
                                     wh_sb[k][:, h * 128:(h + 1) * 128],
                                     xk[:, k, :ncols],
                                     start=(k == 0), stop=(k == 1))
                nc.vector.scalar_tensor_tensor(
                    b_buf[:, h, acol:acol + ncols], yh[:, :ncols],
                    bh_sb[h][:, 0:1], zt[:, :ncols],
                    op0=AluOp.add, op1=AluOp.mult)

        for c in range(NCHUNK):
            a_buf = ab_pool.tile([128, 2, L], F32, name="a_buf", tag="a")
            b_buf = ab_pool.tile([128, 2, L], F32, name="b_buf", tag="b")

            # ---- phase 1: matmuls + sigmoid -> a/b ----
            if c == 0:
                nc.vector.memset(a_buf[:, :, 0:1], 1.0)
                nc.vector.memset(b_buf[:, :, 0:1], 0.0)
                for s in range(4):
                    ncols = SUB if s < 3 else SUB - 1
                    emit_subchunk_mats(s * SUB, ncols, s * SUB + 1)
            else:
                base = c * L - 1
                for s in range(4):
                    emit_subchunk_mats(base + s * SUB, SUB, s * SUB)

            # ---- phase 2: up-sweep (A'-mult on gpsimd, B' ops on DVE) ----
            Aup = lvl_pool.tile([128, 2, LVL_TOTAL], F32, name="Aup", tag="Au")
            Bup = lvl_pool.tile([128, 2, LVL_TOTAL], F32, name="Bup", tag="Bu")
            for lvl in range(LMAX):
                n = L >> lvl
                m = n // 2
                if lvl == 0:
                    sA, sB = a_buf, b_buf
                else:
                    o = LVL_OFF[lvl]
                    sA = Aup[:, :, o:o + n]
                    sB = Bup[:, :, o:o + n]
                o2 = LVL_OFF[lvl + 1]
                dA = Aup[:, :, o2:o2 + m]
                dB = Bup[:, :, o2:o2 + m]
                A_ev, A_od = sA[:, :, 0:n:2], sA[:, :, 1:n:2]
                B_ev, B_od = sB[:, :, 0:n:2], sB[:, :, 1:n:2]
                nc.gpsimd.tensor_tensor(dA, A_ev, B_ev, op=AluOp.mult)
                tu = tmp_pool.tile([128, 2, L // 2], F32, name="tu", tag="tu")
                nc.vector.tensor_tensor(tu[:, :, :m], B_ev, A_od, op=AluOp.mult)
                nc.vector.tensor_tensor(dB, tu[:, :, :m], B_od, op=AluOp.add)

            # ---- phase 3: top-level bookkeeping ----
            o11 = LVL_OFF[LMAX]
            EA = tops_A[:, :, c:c + 1]
            EB = tops_B[:, :, c:c + 1]
            nc.vector.tensor_copy(EA, Aup[:, :, o11:o11 + 1])
            nc.vector.tensor_copy(EB, Bup[:, :, o11:o11 + 1])
            sp = spine
            cc = lambda i: (tops_A[:, :, i:i + 1], tops_B[:, :, i:i + 1])
            if c == 0:
                nc.vector.tensor_copy(otb[:, :, 0:1], EB)
            elif c == 1:
                top_combine(sp[:, :, 0:1], cc(0)[1], *cc(1))
                nc.vector.tensor_copy(otb[:, :, 1:2], sp[:, :, 0:1])
            elif c == 2:
                top_combine(otb[:, :, 2:3], otb[:, :, 1:2], EA, EB)
            elif c == 3:
                top_combine(sp[:, :, 1:2], cc(2)[1], *cc(3))
                nc.vector.tensor_tensor(sp[:, :, 4:5], cc(2)[0], cc(2)[1],
                                        op=AluOp.mult)          # sA12_1
                top_combine(sp[:, :, 7:8], sp[:, :, 0:1],
                            sp[:, :, 4:5], sp[:, :, 1:2])       # sB13_0
                nc.vector.tensor_copy(otb[:, :, 3:4], sp[:, :, 7:8])
            elif c == 4:
                top_combine(otb[:, :, 4:5], otb[:, :, 3:4], EA, EB)
            elif c == 5:
                top_combine(sp[:, :, 2:3], cc(4)[1], *cc(5))    # sB12_2
                nc.vector.tensor_tensor(sp[:, :, 5:6], cc(4)[0], cc(4)[1],
                                        op=AluOp.mult)          # sA12_2
                top_combine(otb[:, :, 5:6], otb[:, :, 3:4],
                            sp[:, :, 5:6], sp[:, :, 2:3])
            elif c == 6:
                top_combine(otb[:, :, 6:7], otb[:, :, 5:6], EA, EB)
            elif c == 7:
                top_combine(sp[:, :, 3:4], cc(6)[1], *cc(7))    # sB12_3
                nc.vector.tensor_tensor(sp[:, :, 6:7], cc(6)[0], cc(6)[1],
                                        op=AluOp.mult)          # sA12_3
                top_combine(sp[:, :, 8:9], sp[:, :, 2:3],
                            sp[:, :, 6:7], sp[:, :, 3:4])       # sB13_1
                nc.vector.tensor_tensor(sp[:, :, 9:10], sp[:, :, 5:6],
                                        sp[:, :, 2:3], op=AluOp.mult)  # sA13_1
                top_combine(sp[:, :, 10:11], sp[:, :, 7:8],
                            sp[:, :, 9:10], sp[:, :, 8:9])      # sB14
                nc.vector.tensor_copy(otb[:, :, 7:8], sp[:, :, 10:11])

            # ---- phase 4: down-sweep into f32r dbuf ----
            dbuf = dbuf_pool.tile([128, 2, L + 1], F32R, name="dbuf", tag="d")
            if c == 0:
                nc.vector.memset(dbuf[:, :, 0:1].bitcast(F32), 0.0)
            else:
                nc.vector.tensor_copy(dbuf[:, :, 0:1], otb[:, :, c - 1:c])
            nc.vector.tensor_copy(dbuf[:, :, L:L + 1], otb[:, :, c:c + 1])
            for lvl in range(LMAX - 1, -1, -1):
                n = L >> lvl
                cnt = n // 2
                step = 1 << (lvl + 1)
                if lvl == 0:
                    A_src, B_src = a_buf, b_buf
                else:
                    o = LVL_OFF[lvl]
                    A_src = Aup[:, :, o:o + n]
                    B_src = Bup[:, :, o:o + n]
                A_ev = A_src[:, :, 0:n:2]
                B_ev = B_src[:, :, 0:n:2]
                Lh = dbuf[:, :, 0:L:step]
                Wt = dbuf[:, :, (1 << lvl):L:step]
                td = tmp_pool.tile([128, 2, L // 2], F32, name="td", tag="td")
                nc.gpsimd.tensor_tensor(td[:, :, :cnt], Lh, A_ev, op=AluOp.mult)
                nc.vector.tensor_tensor(Wt, td[:, :, :cnt], B_ev, op=AluOp.add)

            # ---- phase 5: output matmul straight from dbuf (f32r) ----
            # Always process dbuf cols [1, 2049) = L cols (even-N subchunks,
            # required by f32r matmul).  For chunk 0 the first col is the
            # dummy position-0 value; skip it when storing.
            obase = c * L - 1
            for s in range(4):
                col0 = s * SUB
                po = psum_o.tile([128, 2, SUB], F32, name="po", tag="po")
                for oh in range(2):
                    for k in range(2):
                        nc.tensor.matmul(po[:, oh, :],
                                         wo_sb[k][:, oh * 128:(oh + 1) * 128],
                                         dbuf[:, k, 1 + col0:1 + col0 + SUB],
                                         start=(k == 0), stop=(k == 1))
                osb = osb_pool.tile([128, 2, SUB], F32, name="osb", tag="osb")
                nc.scalar.copy(osb, po)
                skip = 1 if (c == 0 and s == 0) else 0
                dst = out[:, obase + col0 + skip:obase + col0 + SUB]
                nc.sync.dma_start(
                    out=dst.rearrange("(two p) n -> p two n", p=128),
                    in_=osb[:, :, skip:])

            if c == NCHUNK - 1:
                last_dbuf = dbuf

        # ---- final position T: out[p] = out[p-1]*a + b ----
        xl = singles.tile([128, 2, 1], BF, name="xl")
        nc.sync.dma_start(out=xl,
                          in_=xt[:, T - 1:T].rearrange("(k p) n -> p k n", p=128))
        al = singles.tile([128, 2, 1], F32, name="al")
        bl = singles.tile([128, 2, 1], F32, name="bl")
        for h in range(2):
            yzl = psum_y.tile([128, SUB], F32, name="yzl", tag=f"y{h}")[:, 0:1]
            for k in range(2):
                nc.tensor.matmul(yzl, wz_sb[k][:, h * 128:(h + 1) * 128],
                                 xl[:, k, :], start=(k == 0), stop=(k == 1))
            zl = singles.tile([128, 1], F32, name=f"zl{h}")
            nc.scalar.activation(zl, yzl, ActFn.Sigmoid,
                                 bias=bzp_sb[h][:, 0:1], scale=1.0)
            nc.scalar.activation(al[:, h, :], yzl, ActFn.Sigmoid,
                                 bias=bzn_sb[h][:, 0:1], scale=-1.0)
            yhl = psum_y.tile([128, SUB], F32, name="yhl", tag=f"y{h}")[:, 0:1]
            for k in range(2):
                nc.tensor.matmul(yhl, wh_sb[k][:, h * 128:(h + 1) * 128],
                                 xl[:, k, :], start=(k == 0), stop=(k == 1))
            nc.vector.scalar_tensor_tensor(bl[:, h, :], yhl, bh_sb[h][:, 0:1],
                                           zl, op0=AluOp.add, op1=AluOp.mult)
        # f32r matmul needs even N: pad the single final column to 2.
        dl = singles.tile([128, 2, 2], F32R, name="dl")
        tl = singles.tile([128, 2, 1], F32, name="tl")
        nc.vector.memset(dl.bitcast(F32), 0.0)
        nc.vector.tensor_tensor(tl, last_dbuf[:, :, L:L + 1], al, op=AluOp.mult)
        nc.vector.tensor_tensor(dl[:, :, 0:1], tl, bl, op=AluOp.add)
        pol = psum_o.tile([128, 2, SUB], F32, name="pol", tag="po")[:, :, 0:2]
        for oh in range(2):
            for k in range(2):
                nc.tensor.matmul(pol[:, oh, :],
                                 wo_sb[k][:, oh * 128:(oh + 1) * 128],
                                 dl[:, k, :], start=(k == 0), stop=(k == 1))
        osl = singles.tile([128, 2, 1], F32, name="osl")
        nc.scalar.copy(osl, pol[:, :, 0:1])
        nc.sync.dma_start(
            out=out[:, T - 1:T].rearrange("(two p) n -> p two n", p=128),
            in_=osl)

    nc.compile()
    return nc


_NC_CACHE = {}


def _get_nc():
    if "nc" not in _NC_CACHE:
        _NC_CACHE["nc"] = build_nc()
    return _NC_CACHE["nc"]


def _prepare_in_maps(xs, Wz, bz, Wh, bh, Wo, bo):
    xs = np.asarray(xs, np.float32)
    Wz = np.asarray(Wz, np.float32)
    bz = np.asarray(bz, np.float32)
    Wh = np.asarray(Wh, np.float32)
    bh = np.asarray(bh, np.float32)
    Wo = np.asarray(Wo, np.float32)

    wzt = np.ascontiguousarray(Wz.T).astype(BF16)
    wht = np.ascontiguousarray(Wh.T).astype(BF16)
    wot = np.ascontiguousarray(Wo.T)          # f32 bits, f32r on device
    bzp = np.ascontiguousarray(bz.reshape(H, 1))
    bzn = np.ascontiguousarray((-bz).reshape(H, 1))
    bhb = np.ascontiguousarray(bh.reshape(H, 1))

    in_maps = []
    for i in range(B):
        xti = np.ascontiguousarray(xs[i].T).astype(BF16)
        in_maps.append({
            "xt": xti, "wzt": wzt, "wht": wht, "wot": wot,
            "bzp": bzp, "bzn": bzn, "bhb": bhb,
        })
    return in_maps


def _assemble(res, bo):
    bo = np.asarray(bo, np.float32)
    return np.stack([np.asarray(res.results[i]["out"], np.float32).T + bo
                     for i in range(B)], axis=0)


def run_traced(xs, Wz, bz, Wh, bh, Wo, bo, trace=True):
    in_maps = _prepare_in_maps(xs, Wz, bz, Wh, bh, Wo, bo)
    res = run_bass_kernel_spmd(_get_nc(), in_maps, core_ids=list(range(B)),
                               trace=trace)
    return _assemble(res, bo), res


def kernel(xs, Wz, bz, Wh, bh, Wo, bo):
    in_maps = _prepare_in_maps(xs, Wz, bz, Wh, bh, Wo, bo)
    res = run_bass_kernel_spmd(_get_nc(), in_maps, core_ids=list(range(B)))
    return _assemble(res, bo)
